# revision 7
# baseline (speedup 1.0000x reference)
"""Trainium2 Bass kernel for nn_Attention_49993419325755 (per-head LSTM
encoders + masked graph attention), data-parallel over batch on 8 cores.

See bottom of file for the public `kernel(**inputs)` entry point.
"""

import numpy as np

B, S, L, H, D = 32, 325, 192, 8, 128
NCORES = 8
NB = B // NCORES          # batches per core (4)
N = NB * S                # sequences per core (1300)
T = L                     # timesteps (192)
CHUNKS = [(0, 512), (512, 1024), (1024, 1300)]
TT = [(0, 128), (128, 256), (256, 325)]   # t/s tiles of 325
RSQ = 1.0 / np.sqrt(128.0)

_cache = {}


# ----------------------------------------------------------------- device ---
def _build(T_steps=T):
    from tile_patch import apply_patch
    apply_patch()

    import concourse.bass as bass
    import concourse.mybir as mybir
    import concourse.tile as tile

    FP32 = mybir.dt.float32
    FP32R = mybir.dt.float32r
    BF16 = mybir.dt.bfloat16
    AF = mybir.ActivationFunctionType
    ALU = mybir.AluOpType

    nc = bass.Bass()

    def P(name, shape, dt=FP32):
        return nc.declare_dram_parameter(name, shape, dt, isOutput=False)

    x_rep = P("x_rep", [T_steps, 128, N], BF16)
    xv = P("xv", [T_steps, 32, S], BF16)
    whhT_e = P("whhT", [2, 8, 4, 128, 128])
    wih_e = P("wih", [2, 8, 4, 128])
    bias_e = P("bias", [2, 8, 4, 128])
    vw_e = P("vw", [3, 4, 32])
    adjT_e = P("adjT", [3, 128, S], BF16)
    identb_e = P("identb", [128, 128], BF16)
    identf_e = P("identf", [32, 32])
    ones_e = P("ones", [128, 2], BF16)
    out_ext = nc.declare_dram_parameter("out", [NB, S, T_steps, H], FP32, isOutput=True)

    qk_dram = nc.dram_tensor("qk_spill", [2, 8, 128, N], FP32)

    with tile.TileContext(nc) as tc:
      with tc.tile_pool(name="const", bufs=1) as cpool:
        identb = cpool.tile([128, 128], BF16)
        nc.sync.dma_start(identb[:], identb_e[:])
        identf = cpool.tile([32, 32], FP32)
        nc.sync.dma_start(identf[:], identf_e[:])
        onesb = cpool.tile([128, 2], BF16)
        nc.sync.dma_start(onesb[:], ones_e[:])
        # v output store: [node-part, (ttile, h, b, l)] bf16
        v_sb = cpool.tile([128, 3 * 32 * T_steps], BF16)

        # ================= q / k LSTM passes =================
        for pidx in range(2):
          with (
              tc.tile_pool(name="wp", bufs=1) as wp,
              tc.tile_pool(name="wtmp", bufs=2) as wtmp,
              tc.tile_pool(name="state", bufs=1) as statep,
              tc.tile_pool(name="xr", bufs=3) as xrp,
              tc.tile_pool(name="u", bufs=2) as up,
              tc.tile_pool(name="sg", bufs=2) as sgp,
              tc.tile_pool(name="pm", bufs=2) as pmp,
              tc.tile_pool(name="t2", bufs=2) as t2p,
              tc.tile_pool(name="zp", bufs=2, space="PSUM") as zpp,
          ):
            wr = []
            wihb = []
            for c in range(8):
                gw = []
                for g in range(4):
                    wt = wtmp.tile([128, 128], FP32, tag="wt")
                    nc.sync.dma_start(wt[:], whhT_e[pidx, c, g])
                    wrt = wp.tile([128, 128], FP32R, tag=f"wr{c}_{g}")
                    nc.vector.tensor_copy(wrt[:], wt[:])
                    gw.append(wrt)
                wr.append(gw)
                wib = wp.tile([128, 8], FP32, tag=f"wib{c}")
                nc.sync.dma_start(
                    wib[:, 0:4], wih_e[pidx, c].rearrange("g u -> u g"))
                nc.sync.dma_start(
                    wib[:, 4:8], bias_e[pidx, c].rearrange("g u -> u g"))
                wihb.append(wib)

            Ct = []
            ht = []
            for c in range(8):
                Cc = statep.tile([128, N], FP32, tag=f"C{c}")
                nc.vector.memset(Cc[:], 0.0)
                hc = statep.tile([128, N], FP32R, tag=f"h{c}")
                nc.vector.memset(hc[:].bitcast(FP32), 0.0)
                Ct.append(Cc)
                ht.append(hc)

            for t in range(T_steps):
                xr = xrp.tile([128, N], BF16, tag="xr")
                nc.sync.dma_start(xr[:], x_rep[t])
                for c in range(8):
                    u = up.tile([128, 4 * N], BF16, tag="u")
                    for g in range(4):
                        eng = nc.vector if g < 2 else nc.gpsimd
                        eng.tensor_scalar(
                            u[:, g * N:(g + 1) * N], xr[:],
                            wihb[c][:, g:g + 1], wihb[c][:, 4 + g:5 + g],
                            ALU.mult, ALU.add)
                    sg = sgp.tile([128, 4 * N], BF16, tag="sg")
                    sg4 = sg[:].rearrange("p (g x) -> p g x", g=4)
                    for (a0, a1) in CHUNKS:
                        cn = a1 - a0
                        zp = zpp.tile([128, 2048], FP32, tag="zp")
                        zp4 = zp[:].rearrange("p (g x) -> p g x", g=4)
                        for g in range(4):
                            nc.tensor.matmul(
                                zp[:, g * 512:g * 512 + cn], wr[c][g][:],
                                ht[c][:, a0:a1], start=True, stop=False)
                        for g in range(4):
                            nc.tensor.matmul(
                                zp[:, g * 512:g * 512 + cn], identb[:],
                                u[:, g * N + a0:g * N + a1],
                                start=False, stop=True)
                        nc.scalar.activation(
                            sg4[:, :, a0:a1], zp4[:, :, 0:cn], AF.Sigmoid)
                    si = sg[:, 0:N]
                    sf = sg[:, N:2 * N]
                    sgg = sg[:, 2 * N:3 * N]
                    so = sg[:, 3 * N:4 * N]
                    pm = pmp.tile([128, N], BF16, tag="pm")
                    nc.vector.tensor_tensor(pm[:], si, sgg, ALU.mult)
                    m = pmp.tile([128, N], BF16, tag="m")
                    nc.vector.scalar_tensor_tensor(
                        m[:], pm[:], 2.0, si, ALU.mult, ALU.subtract)
                    nc.gpsimd.tensor_tensor(Ct[c][:], Ct[c][:], sf, ALU.mult)
                    nc.vector.tensor_tensor(Ct[c][:], Ct[c][:], m[:], ALU.add)
                    t2 = t2p.tile([128, N], BF16, tag="t2")
                    nc.scalar.activation(t2[:], Ct[c][:], AF.Tanh)
                    nc.vector.tensor_tensor(ht[c][:], so, t2[:], ALU.mult)

            for c in range(8):
                nc.sync.dma_start(
                    qk_dram[pidx, c], ht[c][:].bitcast(FP32))

        # ================= v LSTM =================
        with (
            tc.tile_pool(name="vconst", bufs=1) as vcp,
            tc.tile_pool(name="vstate", bufs=1) as vsp,
            tc.tile_pool(name="vx", bufs=3) as vxp,
            tc.tile_pool(name="vtmp", bufs=3) as vtp,
            tc.tile_pool(name="vps", bufs=2, space="PSUM") as vpsp,
        ):
            vwt = []  # [which][gate] -> [32,1]
            for w in range(3):
                row = []
                for g in range(4):
                    vt = vcp.tile([32, 1], FP32, tag=f"vw{w}_{g}")
                    nc.sync.dma_start(vt[:], vw_e[w, g].rearrange("(j o) -> j o", o=1))
                    row.append(vt)
                vwt.append(row)
            cv = vsp.tile([32, S], FP32)
            nc.vector.memset(cv[:], 0.0)
            hv = vsp.tile([32, S], FP32)
            nc.vector.memset(hv[:], 0.0)
            v_sb5 = v_sb[:].rearrange(
                "p (tt h b l) -> p tt h b l", tt=3, h=8, b=4)

            for t in range(T_steps):
                xvt = vxp.tile([32, S], BF16, tag="xv")
                nc.sync.dma_start(xvt[:], xv[t])
                zg = []
                for g in range(4):
                    xw = vtp.tile([32, S], FP32, tag=f"xw{g}")
                    nc.vector.tensor_scalar(
                        xw[:], xvt[:], vwt[0][g][:], None, ALU.mult)
                    z = vtp.tile([32, S], FP32, tag=f"z{g}")
                    nc.vector.scalar_tensor_tensor(
                        z[:], hv[:], vwt[1][g][:], xw[:], ALU.mult, ALU.add)
                    zg.append(z)
                gi = vtp.tile([32, S], FP32, tag="gi")
                nc.scalar.activation(gi[:], zg[0][:], AF.Sigmoid, bias=vwt[2][0][:])
                gf = vtp.tile([32, S], FP32, tag="gf")
                nc.scalar.activation(gf[:], zg[1][:], AF.Sigmoid, bias=vwt[2][1][:])
                gg = vtp.tile([32, S], FP32, tag="gg")
                nc.scalar.activation(gg[:], zg[2][:], AF.Tanh, bias=vwt[2][2][:])
                go = vtp.tile([32, S], FP32, tag="go")
                nc.scalar.activation(go[:], zg[3][:], AF.Sigmoid, bias=vwt[2][3][:])
                mv = vtp.tile([32, S], FP32, tag="mv")
                nc.vector.tensor_tensor(mv[:], gi[:], gg[:], ALU.mult)
                nc.gpsimd.tensor_tensor(cv[:], cv[:], gf[:], ALU.mult)
                nc.vector.tensor_tensor(cv[:], cv[:], mv[:], ALU.add)
                tv = vtp.tile([32, S], FP32, tag="tv")
                nc.scalar.activation(tv[:], cv[:], AF.Tanh)
                nc.vector.tensor_tensor(hv[:], go[:], tv[:], ALU.mult)
                # transpose hv into v_sb[node, (tt,h,b,l=t)]
                for (ti, (b0, b1)) in enumerate(TT):
                    tl = b1 - b0
                    pt = vpsp.tile([128, 32], FP32, tag="pt")
                    nc.tensor.transpose(
                        pt[0:tl, :], hv[:, b0:b1], identf[:])
                    nc.vector.tensor_copy(
                        v_sb5[0:tl, ti, :, :, t], pt[0:tl, :])

        # ================= attention =================
        with (
            tc.tile_pool(name="adj", bufs=1) as adjp,
            tc.tile_pool(name="qk", bufs=3) as qkp,
            tc.tile_pool(name="em", bufs=2) as emp,
            tc.tile_pool(name="rs", bufs=3) as rsp,
            tc.tile_pool(name="asmp", bufs=2) as asmp,
            tc.tile_pool(name="psS", bufs=2, space="PSUM") as psSp,
            tc.tile_pool(name="psR", bufs=2, space="PSUM") as psRp,
            tc.tile_pool(name="psA", bufs=2, space="PSUM") as psAp,
        ):
            adjt = []
            for ti in range(3):
                at = adjp.tile([128, S], BF16, tag=f"adj{ti}")
                nc.sync.dma_start(at[:], adjT_e[ti])
                adjt.append(at)

            for b in range(NB):
                asms = []
                for (si_, (s0, s1)) in enumerate(TT):
                    at_ = asmp.tile([128, T_steps * H], FP32, tag=f"asm{si_}")
                    asms.append(at_)
                for h in range(8):
                    qhb = qkp.tile([128, 328], FP32R, tag="qhb")
                    nc.vector.memset(qhb[:].bitcast(FP32), 0.0)
                    nc.sync.dma_start(
                        qhb[:, 0:S].bitcast(FP32),
                        qk_dram[0, h, :, b * S:(b + 1) * S])
                    khb = qkp.tile([128, 328], FP32R, tag="khb")
                    nc.vector.memset(khb[:].bitcast(FP32), 0.0)
                    nc.sync.dma_start(
                        khb[:, 0:S].bitcast(FP32),
                        qk_dram[1, h, :, b * S:(b + 1) * S])
                    ems = []
                    for (ti, (t0, t1)) in enumerate(TT):
                        tl = t1 - t0
                        psS = psSp.tile([128, 328], FP32, tag="psS")
                        nc.tensor.matmul(
                            psS[0:tl, :], khb[:, t0:t1], qhb[:],
                            start=True, stop=True)
                        lk = emp.tile([128, S], BF16, tag="lk")
                        nc.scalar.activation(
                            lk[0:tl, :], psS[0:tl, 0:S], AF.Prelu,
                            scale=RSQ, alpha=0.2)
                        em = emp.tile([128, S], BF16, tag=f"em{ti}")
                        nc.scalar.activation(em[0:tl, :], lk[0:tl, :], AF.Exp)
                        nc.vector.tensor_tensor(
                            em[0:tl, :], em[0:tl, :], adjt[ti][0:tl, :],
                            ALU.mult)
                        ems.append(em)
                    for (si_, (s0, s1)) in enumerate(TT):
                        sl = s1 - s0
                        psR = psRp.tile([128, 8], FP32, tag="psR")
                        for (ti, (t0, t1)) in enumerate(TT):
                            tl = t1 - t0
                            nc.tensor.matmul(
                                psR[0:sl, 0:2], ems[ti][0:tl, s0:s1],
                                onesb[0:tl, :],
                                start=(ti == 0), stop=(ti == 2))
                        rs = rsp.tile([128, 1], FP32, tag="rs")
                        nc.vector.reciprocal(rs[0:sl, :], psR[0:sl, 0:1])
                        psA = psAp.tile([128, T_steps], FP32, tag="psA")
                        for (ti, (t0, t1)) in enumerate(TT):
                            tl = t1 - t0
                            nc.tensor.matmul(
                                psA[0:sl, :], ems[ti][0:tl, s0:s1],
                                v_sb5[0:tl, ti, h, b, :],
                                start=(ti == 0), stop=(ti == 2))
                        asm5 = asms[si_][:].rearrange(
                            "p (l hh) -> p l hh", hh=8)
                        nc.scalar.activation(
                            asm5[0:sl, :, h], psA[0:sl, :], AF.Prelu,
                            scale=rs[0:sl, :], alpha=0.2)
                for (si_, (s0, s1)) in enumerate(TT):
                    sl = s1 - s0
                    nc.sync.dma_start(
                        out_ext[b, s0:s1], asms[si_][0:sl, :].rearrange(
                            "p (l hh) -> p l hh", hh=8))

    return nc


# ------------------------------------------------------------------- host ---
def _prep(inputs, T_steps=T):
    import ml_dtypes
    bf16 = ml_dtypes.bfloat16

    x = np.asarray(inputs["x"], np.float32)          # [B,S,L,1]
    graph = np.asarray(inputs["graph"], np.float32)  # [S,S]

    shared = {}
    whhT = np.zeros((2, 8, 4, 128, 128), np.float32)
    wih = np.zeros((2, 8, 4, 128), np.float32)
    bias = np.zeros((2, 8, 4, 128), np.float32)
    for pidx, pre in enumerate(("q", "k")):
        W_ih = np.asarray(inputs[f"{pre}_Wih"], np.float32)   # [8,512,1]
        W_hh = np.asarray(inputs[f"{pre}_Whh"], np.float32)   # [8,512,128]
        b_ = (np.asarray(inputs[f"{pre}_bih"], np.float32)
              + np.asarray(inputs[f"{pre}_bhh"], np.float32))  # [8,512]
        for h in range(8):
            for g in range(4):
                sc = 2.0 if g == 2 else 1.0
                whhT[pidx, h, g] = sc * W_hh[h, g * 128:(g + 1) * 128, :].T
                wih[pidx, h, g] = sc * W_ih[h, g * 128:(g + 1) * 128, 0]
                bias[pidx, h, g] = sc * b_[h, g * 128:(g + 1) * 128]
    shared["whhT"] = whhT
    shared["wih"] = wih
    shared["bias"] = bias

    vW_ih = np.asarray(inputs["v_Wih"], np.float32)[:, :, 0]  # [8,4]
    vW_hh = np.asarray(inputs["v_Whh"], np.float32)[:, :, 0]  # [8,4]
    vb = (np.asarray(inputs["v_bih"], np.float32)
          + np.asarray(inputs["v_bhh"], np.float32))          # [8,4]
    vw = np.zeros((3, 4, 32), np.float32)
    for h in range(8):
        for b in range(NB):
            j = h * NB + b
            vw[0, :, j] = vW_ih[h]
            vw[1, :, j] = vW_hh[h]
            vw[2, :, j] = vb[h]
    shared["vw"] = vw

    A = ((graph + np.eye(S, dtype=np.float32)) != 0).astype(np.float32)
    adjT = np.zeros((3, 128, S), np.float32)
    for ti, (t0, t1) in enumerate(TT):
        adjT[ti, 0:t1 - t0] = A[t0:t1, :]
    shared["adjT"] = adjT.astype(bf16)
    shared["identb"] = np.eye(128, dtype=np.float32).astype(bf16)
    shared["identf"] = np.eye(32, dtype=np.float32)
    shared["ones"] = np.ones((128, 2), np.float32).astype(bf16)

    in_maps = []
    for core in range(NCORES):
        xc = x[core * NB:(core + 1) * NB, :, :, 0]   # [NB,S,L]
        xt = xc.transpose(2, 0, 1).reshape(T, N)[:T_steps]     # [T,N]
        x_rep = np.ascontiguousarray(
            np.broadcast_to(xt[:, None, :], (T_steps, 128, N))).astype(bf16)
        xvv = np.ascontiguousarray(
            np.broadcast_to(
                xc.transpose(2, 0, 1)[:T_steps, None, :, :],
                (T_steps, 8, NB, S)).reshape(T_steps, 32, S)).astype(bf16)
        m = dict(shared)
        m["x_rep"] = x_rep
        m["xv"] = xvv
        in_maps.append(m)
    return in_maps


def _run(inputs, T_steps=T, trace=False):
    import sys
    if "/root/problem" not in sys.path:
        sys.path.insert(0, "/root/problem")
    from concourse.bass_utils import run_bass_kernel_spmd

    key = T_steps
    if key not in _cache:
        _cache[key] = _build(T_steps)
    nc = _cache[key]
    in_maps = _prep(inputs, T_steps)
    res = run_bass_kernel_spmd(
        nc, in_maps, core_ids=list(range(NCORES)), trace=trace)
    out = np.concatenate([res.results[i]["out"] for i in range(NCORES)], axis=0)
    return out, res


def kernel(**inputs):
    out, _ = _run(inputs)
    return out.astype(np.float32)


# revision 8
# speedup vs baseline: 1.0709x; 1.0709x over previous
"""Trainium2 Bass kernel for nn_Attention_49993419325755 (per-head LSTM
encoders + masked graph attention), data-parallel over batch on 8 cores.

See bottom of file for the public `kernel(**inputs)` entry point.
"""

import numpy as np

B, S, L, H, D = 32, 325, 192, 8, 128
NCORES = 8
NB = B // NCORES          # batches per core (4)
N = NB * S                # sequences per core (1300)
T = L                     # timesteps (192)
CHUNKS = [(0, 512), (512, 1024), (1024, 1300)]
TT = [(0, 128), (128, 256), (256, 325)]   # t/s tiles of 325
RSQ = 1.0 / np.sqrt(128.0)

_cache = {}


# ----------------------------------------------------------------- device ---
def _build(T_steps=T):
    from tile_patch import apply_patch
    apply_patch()

    import concourse.bass as bass
    import concourse.mybir as mybir
    import concourse.tile as tile

    FP32 = mybir.dt.float32
    FP32R = mybir.dt.float32r
    BF16 = mybir.dt.bfloat16
    AF = mybir.ActivationFunctionType
    ALU = mybir.AluOpType

    nc = bass.Bass()

    def P(name, shape, dt=FP32):
        return nc.declare_dram_parameter(name, shape, dt, isOutput=False)

    x_rep = P("x_rep", [T_steps, 128, N], BF16)
    xv = P("xv", [T_steps, 32, S], BF16)
    whhT_e = P("whhT", [2, 8, 4, 128, 128])
    wih_e = P("wih", [2, 8, 4, 128])
    bias_e = P("bias", [2, 8, 4, 128])
    vw_e = P("vw", [3, 4, 32])
    adjT_e = P("adjT", [3, 128, S], BF16)
    identb_e = P("identb", [128, 128], BF16)
    identf_e = P("identf", [32, 32])
    ones_e = P("ones", [128, 2], BF16)
    out_ext = nc.declare_dram_parameter("out", [NB, S, T_steps, H], FP32, isOutput=True)

    qk_dram = nc.dram_tensor("qk_spill", [2, 8, 128, N], FP32)

    with tile.TileContext(nc) as tc:
      with tc.tile_pool(name="const", bufs=1) as cpool:
        identb = cpool.tile([128, 128], BF16)
        nc.sync.dma_start(identb[:], identb_e[:])
        identf = cpool.tile([32, 32], FP32)
        nc.sync.dma_start(identf[:], identf_e[:])
        onesb = cpool.tile([128, 2], BF16)
        nc.sync.dma_start(onesb[:], ones_e[:])
        # v output store: [node-part, (ttile, h, b, l)] bf16
        v_sb = cpool.tile([128, 3 * 32 * T_steps], BF16)

        # ================= q / k LSTM passes =================
        for pidx in range(2):
          with (
              tc.tile_pool(name="wp", bufs=1) as wp,
              tc.tile_pool(name="wtmp", bufs=2) as wtmp,
              tc.tile_pool(name="state", bufs=1) as statep,
              tc.tile_pool(name="xr", bufs=3) as xrp,
              tc.tile_pool(name="u", bufs=2) as up,
              tc.tile_pool(name="sg", bufs=2) as sgp,
              tc.tile_pool(name="pm", bufs=2) as pmp,
              tc.tile_pool(name="t2", bufs=2) as t2p,
              tc.tile_pool(name="zp", bufs=2, space="PSUM") as zpp,
          ):
            wr = []
            wihb = []
            for c in range(8):
                gw = []
                for g in range(4):
                    wt = wtmp.tile([128, 128], FP32, tag="wt")
                    nc.sync.dma_start(wt[:], whhT_e[pidx, c, g])
                    wrt = wp.tile([128, 128], FP32R, tag=f"wr{c}_{g}")
                    nc.vector.tensor_copy(wrt[:], wt[:])
                    gw.append(wrt)
                wr.append(gw)
                wib = wp.tile([128, 8], FP32, tag=f"wib{c}")
                nc.sync.dma_start(
                    wib[:, 0:4], wih_e[pidx, c].rearrange("g u -> u g"))
                nc.sync.dma_start(
                    wib[:, 4:8], bias_e[pidx, c].rearrange("g u -> u g"))
                wihb.append(wib)

            Ct = []
            ht = []
            for c in range(8):
                Cc = statep.tile([128, N], BF16, tag=f"C{c}")
                nc.vector.memset(Cc[:], 0.0)
                hc = statep.tile([128, N], FP32R, tag=f"h{c}")
                nc.vector.memset(hc[:].bitcast(FP32), 0.0)
                Ct.append(Cc)
                ht.append(hc)

            for t in range(T_steps):
                xr = xrp.tile([128, N], BF16, tag="xr")
                nc.sync.dma_start(xr[:], x_rep[t])
                for c in range(8):
                    u = up.tile([128, 4 * N], BF16, tag="u")
                    for g in range(4):
                        eng = nc.vector if g < 3 else nc.gpsimd
                        eng.tensor_scalar(
                            u[:, g * N:(g + 1) * N], xr[:],
                            wihb[c][:, g:g + 1], wihb[c][:, 4 + g:5 + g],
                            ALU.mult, ALU.add)
                    sg = sgp.tile([128, 4 * N], BF16, tag="sg")
                    sg4 = sg[:].rearrange("p (g x) -> p g x", g=4)
                    for (a0, a1) in CHUNKS:
                        cn = a1 - a0
                        zp = zpp.tile([128, 2048], FP32, tag="zp")
                        zp4 = zp[:].rearrange("p (g x) -> p g x", g=4)
                        for g in range(4):
                            nc.tensor.matmul(
                                zp[:, g * 512:g * 512 + cn], wr[c][g][:],
                                ht[c][:, a0:a1], start=True, stop=False)
                        for g in range(4):
                            nc.tensor.matmul(
                                zp[:, g * 512:g * 512 + cn], identb[:],
                                u[:, g * N + a0:g * N + a1],
                                start=False, stop=True)
                        nc.scalar.activation(
                            sg4[:, :, a0:a1], zp4[:, :, 0:cn], AF.Sigmoid)
                    si = sg[:, 0:N]
                    sf = sg[:, N:2 * N]
                    sgg = sg[:, 2 * N:3 * N]
                    so = sg[:, 3 * N:4 * N]
                    g2 = pmp.tile([128, N], BF16, tag="g2")
                    nc.vector.tensor_scalar(
                        g2[:], sgg, 2.0, -1.0, ALU.mult, ALU.add)
                    m = pmp.tile([128, N], BF16, tag="m")
                    nc.vector.tensor_tensor(m[:], si, g2[:], ALU.mult)
                    nc.gpsimd.tensor_tensor(Ct[c][:], Ct[c][:], sf, ALU.mult)
                    nc.vector.tensor_tensor(Ct[c][:], Ct[c][:], m[:], ALU.add)
                    t2 = t2p.tile([128, N], BF16, tag="t2")
                    nc.scalar.activation(t2[:], Ct[c][:], AF.Tanh)
                    nc.vector.tensor_tensor(ht[c][:], so, t2[:], ALU.mult)

            for c in range(8):
                nc.sync.dma_start(
                    qk_dram[pidx, c], ht[c][:].bitcast(FP32))

        # ================= v LSTM =================
        with (
            tc.tile_pool(name="vconst", bufs=1) as vcp,
            tc.tile_pool(name="vstate", bufs=1) as vsp,
            tc.tile_pool(name="vx", bufs=3) as vxp,
            tc.tile_pool(name="vtmp", bufs=3) as vtp,
            tc.tile_pool(name="vps", bufs=2, space="PSUM") as vpsp,
        ):
            vwt = []  # [which][gate] -> [32,1]
            for w in range(3):
                row = []
                for g in range(4):
                    vt = vcp.tile([32, 1], FP32, tag=f"vw{w}_{g}")
                    nc.sync.dma_start(vt[:], vw_e[w, g].rearrange("(j o) -> j o", o=1))
                    row.append(vt)
                vwt.append(row)
            cv = vsp.tile([32, S], FP32)
            nc.vector.memset(cv[:], 0.0)
            hv = vsp.tile([32, S], FP32)
            nc.vector.memset(hv[:], 0.0)
            v_sb5 = v_sb[:].rearrange(
                "p (tt h b l) -> p tt h b l", tt=3, h=8, b=4)

            for t in range(T_steps):
                xvt = vxp.tile([32, S], BF16, tag="xv")
                nc.sync.dma_start(xvt[:], xv[t])
                zg = []
                for g in range(4):
                    xw = vtp.tile([32, S], FP32, tag=f"xw{g}")
                    nc.vector.tensor_scalar(
                        xw[:], xvt[:], vwt[0][g][:], None, ALU.mult)
                    z = vtp.tile([32, S], FP32, tag=f"z{g}")
                    nc.vector.scalar_tensor_tensor(
                        z[:], hv[:], vwt[1][g][:], xw[:], ALU.mult, ALU.add)
                    zg.append(z)
                gi = vtp.tile([32, S], FP32, tag="gi")
                nc.scalar.activation(gi[:], zg[0][:], AF.Sigmoid, bias=vwt[2][0][:])
                gf = vtp.tile([32, S], FP32, tag="gf")
                nc.scalar.activation(gf[:], zg[1][:], AF.Sigmoid, bias=vwt[2][1][:])
                gg = vtp.tile([32, S], FP32, tag="gg")
                nc.scalar.activation(gg[:], zg[2][:], AF.Tanh, bias=vwt[2][2][:])
                go = vtp.tile([32, S], FP32, tag="go")
                nc.scalar.activation(go[:], zg[3][:], AF.Sigmoid, bias=vwt[2][3][:])
                mv = vtp.tile([32, S], FP32, tag="mv")
                nc.vector.tensor_tensor(mv[:], gi[:], gg[:], ALU.mult)
                nc.gpsimd.tensor_tensor(cv[:], cv[:], gf[:], ALU.mult)
                nc.vector.tensor_tensor(cv[:], cv[:], mv[:], ALU.add)
                tv = vtp.tile([32, S], FP32, tag="tv")
                nc.scalar.activation(tv[:], cv[:], AF.Tanh)
                nc.vector.tensor_tensor(hv[:], go[:], tv[:], ALU.mult)
                # transpose hv into v_sb[node, (tt,h,b,l=t)]
                for (ti, (b0, b1)) in enumerate(TT):
                    tl = b1 - b0
                    pt = vpsp.tile([128, 32], FP32, tag="pt")
                    nc.tensor.transpose(
                        pt[0:tl, :], hv[:, b0:b1], identf[:])
                    nc.vector.tensor_copy(
                        v_sb5[0:tl, ti, :, :, t], pt[0:tl, :])

        # ================= attention =================
        with (
            tc.tile_pool(name="adj", bufs=1) as adjp,
            tc.tile_pool(name="qk", bufs=3) as qkp,
            tc.tile_pool(name="em", bufs=2) as emp,
            tc.tile_pool(name="rs", bufs=3) as rsp,
            tc.tile_pool(name="asmp", bufs=2) as asmp,
            tc.tile_pool(name="psS", bufs=2, space="PSUM") as psSp,
            tc.tile_pool(name="psR", bufs=2, space="PSUM") as psRp,
            tc.tile_pool(name="psA", bufs=2, space="PSUM") as psAp,
        ):
            adjt = []
            for ti in range(3):
                at = adjp.tile([128, S], BF16, tag=f"adj{ti}")
                nc.sync.dma_start(at[:], adjT_e[ti])
                adjt.append(at)

            for b in range(NB):
                asms = []
                for (si_, (s0, s1)) in enumerate(TT):
                    at_ = asmp.tile([128, T_steps * H], FP32, tag=f"asm{si_}")
                    asms.append(at_)
                for h in range(8):
                    qhb = qkp.tile([128, 328], FP32R, tag="qhb")
                    nc.vector.memset(qhb[:].bitcast(FP32), 0.0)
                    nc.sync.dma_start(
                        qhb[:, 0:S].bitcast(FP32),
                        qk_dram[0, h, :, b * S:(b + 1) * S])
                    khb = qkp.tile([128, 328], FP32R, tag="khb")
                    nc.vector.memset(khb[:].bitcast(FP32), 0.0)
                    nc.sync.dma_start(
                        khb[:, 0:S].bitcast(FP32),
                        qk_dram[1, h, :, b * S:(b + 1) * S])
                    ems = []
                    for (ti, (t0, t1)) in enumerate(TT):
                        tl = t1 - t0
                        psS = psSp.tile([128, 328], FP32, tag="psS")
                        nc.tensor.matmul(
                            psS[0:tl, :], khb[:, t0:t1], qhb[:],
                            start=True, stop=True)
                        lk = emp.tile([128, S], BF16, tag="lk")
                        nc.scalar.activation(
                            lk[0:tl, :], psS[0:tl, 0:S], AF.Prelu,
                            scale=RSQ, alpha=0.2)
                        em = emp.tile([128, S], BF16, tag=f"em{ti}")
                        nc.scalar.activation(em[0:tl, :], lk[0:tl, :], AF.Exp)
                        nc.vector.tensor_tensor(
                            em[0:tl, :], em[0:tl, :], adjt[ti][0:tl, :],
                            ALU.mult)
                        ems.append(em)
                    for (si_, (s0, s1)) in enumerate(TT):
                        sl = s1 - s0
                        psR = psRp.tile([128, 8], FP32, tag="psR")
                        for (ti, (t0, t1)) in enumerate(TT):
                            tl = t1 - t0
                            nc.tensor.matmul(
                                psR[0:sl, 0:2], ems[ti][0:tl, s0:s1],
                                onesb[0:tl, :],
                                start=(ti == 0), stop=(ti == 2))
                        rs = rsp.tile([128, 1], FP32, tag="rs")
                        nc.vector.reciprocal(rs[0:sl, :], psR[0:sl, 0:1])
                        psA = psAp.tile([128, T_steps], FP32, tag="psA")
                        for (ti, (t0, t1)) in enumerate(TT):
                            tl = t1 - t0
                            nc.tensor.matmul(
                                psA[0:sl, :], ems[ti][0:tl, s0:s1],
                                v_sb5[0:tl, ti, h, b, :],
                                start=(ti == 0), stop=(ti == 2))
                        asm5 = asms[si_][:].rearrange(
                            "p (l hh) -> p l hh", hh=8)
                        nc.scalar.activation(
                            asm5[0:sl, :, h], psA[0:sl, :], AF.Prelu,
                            scale=rs[0:sl, :], alpha=0.2)
                for (si_, (s0, s1)) in enumerate(TT):
                    sl = s1 - s0
                    nc.sync.dma_start(
                        out_ext[b, s0:s1], asms[si_][0:sl, :].rearrange(
                            "p (l hh) -> p l hh", hh=8))

    return nc


# ------------------------------------------------------------------- host ---
def _prep(inputs, T_steps=T):
    import ml_dtypes
    bf16 = ml_dtypes.bfloat16

    x = np.asarray(inputs["x"], np.float32)          # [B,S,L,1]
    graph = np.asarray(inputs["graph"], np.float32)  # [S,S]

    shared = {}
    whhT = np.zeros((2, 8, 4, 128, 128), np.float32)
    wih = np.zeros((2, 8, 4, 128), np.float32)
    bias = np.zeros((2, 8, 4, 128), np.float32)
    for pidx, pre in enumerate(("q", "k")):
        W_ih = np.asarray(inputs[f"{pre}_Wih"], np.float32)   # [8,512,1]
        W_hh = np.asarray(inputs[f"{pre}_Whh"], np.float32)   # [8,512,128]
        b_ = (np.asarray(inputs[f"{pre}_bih"], np.float32)
              + np.asarray(inputs[f"{pre}_bhh"], np.float32))  # [8,512]
        for h in range(8):
            for g in range(4):
                sc = 2.0 if g == 2 else 1.0
                whhT[pidx, h, g] = sc * W_hh[h, g * 128:(g + 1) * 128, :].T
                wih[pidx, h, g] = sc * W_ih[h, g * 128:(g + 1) * 128, 0]
                bias[pidx, h, g] = sc * b_[h, g * 128:(g + 1) * 128]
    shared["whhT"] = whhT
    shared["wih"] = wih
    shared["bias"] = bias

    vW_ih = np.asarray(inputs["v_Wih"], np.float32)[:, :, 0]  # [8,4]
    vW_hh = np.asarray(inputs["v_Whh"], np.float32)[:, :, 0]  # [8,4]
    vb = (np.asarray(inputs["v_bih"], np.float32)
          + np.asarray(inputs["v_bhh"], np.float32))          # [8,4]
    vw = np.zeros((3, 4, 32), np.float32)
    for h in range(8):
        for b in range(NB):
            j = h * NB + b
            vw[0, :, j] = vW_ih[h]
            vw[1, :, j] = vW_hh[h]
            vw[2, :, j] = vb[h]
    shared["vw"] = vw

    A = ((graph + np.eye(S, dtype=np.float32)) != 0).astype(np.float32)
    adjT = np.zeros((3, 128, S), np.float32)
    for ti, (t0, t1) in enumerate(TT):
        adjT[ti, 0:t1 - t0] = A[t0:t1, :]
    shared["adjT"] = adjT.astype(bf16)
    shared["identb"] = np.eye(128, dtype=np.float32).astype(bf16)
    shared["identf"] = np.eye(32, dtype=np.float32)
    shared["ones"] = np.ones((128, 2), np.float32).astype(bf16)

    in_maps = []
    for core in range(NCORES):
        xc = x[core * NB:(core + 1) * NB, :, :, 0]   # [NB,S,L]
        xt = xc.transpose(2, 0, 1).reshape(T, N)[:T_steps]     # [T,N]
        x_rep = np.ascontiguousarray(
            np.broadcast_to(xt[:, None, :], (T_steps, 128, N))).astype(bf16)
        xvv = np.ascontiguousarray(
            np.broadcast_to(
                xc.transpose(2, 0, 1)[:T_steps, None, :, :],
                (T_steps, 8, NB, S)).reshape(T_steps, 32, S)).astype(bf16)
        m = dict(shared)
        m["x_rep"] = x_rep
        m["xv"] = xvv
        in_maps.append(m)
    return in_maps


def _run(inputs, T_steps=T, trace=False):
    import sys
    if "/root/problem" not in sys.path:
        sys.path.insert(0, "/root/problem")
    from concourse.bass_utils import run_bass_kernel_spmd

    key = T_steps
    if key not in _cache:
        _cache[key] = _build(T_steps)
    nc = _cache[key]
    in_maps = _prep(inputs, T_steps)
    res = run_bass_kernel_spmd(
        nc, in_maps, core_ids=list(range(NCORES)), trace=trace)
    out = np.concatenate([res.results[i]["out"] for i in range(NCORES)], axis=0)
    return out, res


def kernel(**inputs):
    out, _ = _run(inputs)
    return out.astype(np.float32)


# revision 9
# speedup vs baseline: 1.1600x; 1.0832x over previous
"""Trainium2 Bass kernel for nn_Attention_49993419325755 (per-head LSTM
encoders + masked graph attention), data-parallel over batch on 8 cores.

See bottom of file for the public `kernel(**inputs)` entry point.
"""

import numpy as np

B, S, L, H, D = 32, 325, 192, 8, 128
NCORES = 8
NB = B // NCORES          # batches per core (4)
N = NB * S                # sequences per core (1300)
T = L                     # timesteps (192)
CHUNKS = [(0, 512), (512, 1024), (1024, 1300)]
TT = [(0, 128), (128, 256), (256, 325)]   # t/s tiles of 325
RSQ = 1.0 / np.sqrt(128.0)

_cache = {}


# ----------------------------------------------------------------- device ---
def _build(T_steps=T):
    from tile_patch import apply_patch
    apply_patch()

    import concourse.bass as bass
    import concourse.mybir as mybir
    import concourse.tile as tile

    FP32 = mybir.dt.float32
    FP32R = mybir.dt.float32r
    BF16 = mybir.dt.bfloat16
    AF = mybir.ActivationFunctionType
    ALU = mybir.AluOpType

    nc = bass.Bass()

    def P(name, shape, dt=FP32):
        return nc.declare_dram_parameter(name, shape, dt, isOutput=False)

    x_rep = P("x_rep", [T_steps, 128, N], BF16)
    xv = P("xv", [T_steps, 32, S], BF16)
    whhT_e = P("whhT", [2, 8, 4, 128, 128])
    wih_e = P("wih", [2, 8, 4, 128])
    bias_e = P("bias", [2, 8, 4, 128])
    vw_e = P("vw", [3, 4, 32])
    adjT_e = P("adjT", [3, 128, S], BF16)
    identb_e = P("identb", [128, 128], BF16)
    identf_e = P("identf", [32, 32])
    ones_e = P("ones", [128, 2], BF16)
    out_ext = nc.declare_dram_parameter("out", [NB, S, T_steps, H], FP32, isOutput=True)

    qk_dram = nc.dram_tensor("qk_spill", [2, 8, 128, N], BF16)

    with tile.TileContext(nc) as tc:
      with tc.tile_pool(name="const", bufs=1) as cpool:
        identb = cpool.tile([128, 128], BF16)
        nc.sync.dma_start(identb[:], identb_e[:])
        identf = cpool.tile([32, 32], FP32)
        nc.sync.dma_start(identf[:], identf_e[:])
        onesb = cpool.tile([128, 2], BF16)
        nc.sync.dma_start(onesb[:], ones_e[:])
        # v output store: [node-part, (ttile, h, b, l)] bf16
        v_sb = cpool.tile([128, 3 * 32 * T_steps], BF16)

        # ================= q / k LSTM passes =================
        for pidx in range(2):
          with (
              tc.tile_pool(name="wp", bufs=1) as wp,
              tc.tile_pool(name="wtmp", bufs=2) as wtmp,
              tc.tile_pool(name="state", bufs=1) as statep,
              tc.tile_pool(name="xr", bufs=3) as xrp,
              tc.tile_pool(name="u", bufs=2) as up,
              tc.tile_pool(name="sg", bufs=2) as sgp,
              tc.tile_pool(name="pm", bufs=2) as pmp,
              tc.tile_pool(name="t2", bufs=2) as t2p,
              tc.tile_pool(name="zp", bufs=2, space="PSUM") as zpp,
          ):
            wr = []
            wihb = []
            for c in range(8):
                gw = []
                for g in range(4):
                    wt = wtmp.tile([128, 128], FP32, tag="wt")
                    nc.sync.dma_start(wt[:], whhT_e[pidx, c, g])
                    wrt = wp.tile([128, 128], BF16, tag=f"wr{c}_{g}")
                    nc.vector.tensor_copy(wrt[:], wt[:])
                    gw.append(wrt)
                wr.append(gw)
                wib = wp.tile([128, 8], FP32, tag=f"wib{c}")
                nc.sync.dma_start(
                    wib[:, 0:4], wih_e[pidx, c].rearrange("g u -> u g"))
                nc.sync.dma_start(
                    wib[:, 4:8], bias_e[pidx, c].rearrange("g u -> u g"))
                wihb.append(wib)

            Ct = []
            ht = []
            for c in range(8):
                Cc = statep.tile([128, N], BF16, tag=f"C{c}")
                nc.vector.memset(Cc[:], 0.0)
                hc = statep.tile([128, N], BF16, tag=f"h{c}")
                nc.vector.memset(hc[:], 0.0)
                Ct.append(Cc)
                ht.append(hc)

            for t in range(T_steps):
                xr = xrp.tile([128, N], BF16, tag="xr")
                nc.sync.dma_start(xr[:], x_rep[t])
                for c in range(8):
                    u = up.tile([128, 4 * N], BF16, tag="u")
                    for g in range(4):
                        eng = nc.vector if g < 3 else nc.gpsimd
                        eng.tensor_scalar(
                            u[:, g * N:(g + 1) * N], xr[:],
                            wihb[c][:, g:g + 1], wihb[c][:, 4 + g:5 + g],
                            ALU.mult, ALU.add)
                    sg = sgp.tile([128, 4 * N], BF16, tag="sg")
                    sg4 = sg[:].rearrange("p (g x) -> p g x", g=4)
                    for (a0, a1) in CHUNKS:
                        cn = a1 - a0
                        zp = zpp.tile([128, 2048], FP32, tag="zp")
                        zp4 = zp[:].rearrange("p (g x) -> p g x", g=4)
                        for g in range(4):
                            nc.tensor.matmul(
                                zp[:, g * 512:g * 512 + cn], wr[c][g][:],
                                ht[c][:, a0:a1], start=True, stop=False)
                        for g in range(4):
                            nc.tensor.matmul(
                                zp[:, g * 512:g * 512 + cn], identb[:],
                                u[:, g * N + a0:g * N + a1],
                                start=False, stop=True)
                        nc.scalar.activation(
                            sg4[:, :, a0:a1], zp4[:, :, 0:cn], AF.Sigmoid)
                    si = sg[:, 0:N]
                    sf = sg[:, N:2 * N]
                    sgg = sg[:, 2 * N:3 * N]
                    so = sg[:, 3 * N:4 * N]
                    g2 = pmp.tile([128, N], BF16, tag="g2")
                    nc.vector.tensor_scalar(
                        g2[:], sgg, 2.0, -1.0, ALU.mult, ALU.add)
                    m = pmp.tile([128, N], BF16, tag="m")
                    nc.vector.tensor_tensor(m[:], si, g2[:], ALU.mult)
                    nc.gpsimd.tensor_tensor(Ct[c][:], Ct[c][:], sf, ALU.mult)
                    nc.vector.tensor_tensor(Ct[c][:], Ct[c][:], m[:], ALU.add)
                    t2 = t2p.tile([128, N], BF16, tag="t2")
                    nc.scalar.activation(t2[:], Ct[c][:], AF.Tanh)
                    nc.vector.tensor_tensor(ht[c][:], so, t2[:], ALU.mult)

            for c in range(8):
                nc.sync.dma_start(qk_dram[pidx, c], ht[c][:])

        # ================= v LSTM =================
        with (
            tc.tile_pool(name="vconst", bufs=1) as vcp,
            tc.tile_pool(name="vstate", bufs=1) as vsp,
            tc.tile_pool(name="vx", bufs=3) as vxp,
            tc.tile_pool(name="vtmp", bufs=3) as vtp,
            tc.tile_pool(name="vps", bufs=2, space="PSUM") as vpsp,
        ):
            vwt = []  # [which][gate] -> [32,1]
            for w in range(3):
                row = []
                for g in range(4):
                    vt = vcp.tile([32, 1], FP32, tag=f"vw{w}_{g}")
                    nc.sync.dma_start(vt[:], vw_e[w, g].rearrange("(j o) -> j o", o=1))
                    row.append(vt)
                vwt.append(row)
            cv = vsp.tile([32, S], FP32)
            nc.vector.memset(cv[:], 0.0)
            hv = vsp.tile([32, S], FP32)
            nc.vector.memset(hv[:], 0.0)
            v_sb5 = v_sb[:].rearrange(
                "p (tt h b l) -> p tt h b l", tt=3, h=8, b=4)

            for t in range(T_steps):
                xvt = vxp.tile([32, S], BF16, tag="xv")
                nc.sync.dma_start(xvt[:], xv[t])
                zg = []
                for g in range(4):
                    xw = vtp.tile([32, S], FP32, tag=f"xw{g}")
                    nc.vector.tensor_scalar(
                        xw[:], xvt[:], vwt[0][g][:], None, ALU.mult)
                    z = vtp.tile([32, S], FP32, tag=f"z{g}")
                    nc.vector.scalar_tensor_tensor(
                        z[:], hv[:], vwt[1][g][:], xw[:], ALU.mult, ALU.add)
                    zg.append(z)
                gi = vtp.tile([32, S], FP32, tag="gi")
                nc.scalar.activation(gi[:], zg[0][:], AF.Sigmoid, bias=vwt[2][0][:])
                gf = vtp.tile([32, S], FP32, tag="gf")
                nc.scalar.activation(gf[:], zg[1][:], AF.Sigmoid, bias=vwt[2][1][:])
                gg = vtp.tile([32, S], FP32, tag="gg")
                nc.scalar.activation(gg[:], zg[2][:], AF.Tanh, bias=vwt[2][2][:])
                go = vtp.tile([32, S], FP32, tag="go")
                nc.scalar.activation(go[:], zg[3][:], AF.Sigmoid, bias=vwt[2][3][:])
                mv = vtp.tile([32, S], FP32, tag="mv")
                nc.vector.tensor_tensor(mv[:], gi[:], gg[:], ALU.mult)
                nc.gpsimd.tensor_tensor(cv[:], cv[:], gf[:], ALU.mult)
                nc.vector.tensor_tensor(cv[:], cv[:], mv[:], ALU.add)
                tv = vtp.tile([32, S], FP32, tag="tv")
                nc.scalar.activation(tv[:], cv[:], AF.Tanh)
                nc.vector.tensor_tensor(hv[:], go[:], tv[:], ALU.mult)
                # transpose hv into v_sb[node, (tt,h,b,l=t)]
                for (ti, (b0, b1)) in enumerate(TT):
                    tl = b1 - b0
                    pt = vpsp.tile([128, 32], FP32, tag="pt")
                    nc.tensor.transpose(
                        pt[0:tl, :], hv[:, b0:b1], identf[:])
                    nc.vector.tensor_copy(
                        v_sb5[0:tl, ti, :, :, t], pt[0:tl, :])

        # ================= attention =================
        with (
            tc.tile_pool(name="adj", bufs=1) as adjp,
            tc.tile_pool(name="qk", bufs=3) as qkp,
            tc.tile_pool(name="em", bufs=2) as emp,
            tc.tile_pool(name="rs", bufs=3) as rsp,
            tc.tile_pool(name="asmp", bufs=2) as asmp,
            tc.tile_pool(name="psS", bufs=2, space="PSUM") as psSp,
            tc.tile_pool(name="psR", bufs=2, space="PSUM") as psRp,
            tc.tile_pool(name="psA", bufs=2, space="PSUM") as psAp,
        ):
            adjt = []
            for ti in range(3):
                at = adjp.tile([128, S], BF16, tag=f"adj{ti}")
                nc.sync.dma_start(at[:], adjT_e[ti])
                adjt.append(at)

            for b in range(NB):
                asms = []
                for (si_, (s0, s1)) in enumerate(TT):
                    at_ = asmp.tile([128, T_steps * H], FP32, tag=f"asm{si_}")
                    asms.append(at_)
                for h in range(8):
                    qhb = qkp.tile([128, 328], BF16, tag="qhb")
                    nc.vector.memset(qhb[:], 0.0)
                    nc.sync.dma_start(
                        qhb[:, 0:S], qk_dram[0, h, :, b * S:(b + 1) * S])
                    khb = qkp.tile([128, 328], BF16, tag="khb")
                    nc.vector.memset(khb[:], 0.0)
                    nc.sync.dma_start(
                        khb[:, 0:S], qk_dram[1, h, :, b * S:(b + 1) * S])
                    ems = []
                    for (ti, (t0, t1)) in enumerate(TT):
                        tl = t1 - t0
                        psS = psSp.tile([128, 328], FP32, tag="psS")
                        nc.tensor.matmul(
                            psS[0:tl, :], khb[:, t0:t1], qhb[:],
                            start=True, stop=True)
                        lk = emp.tile([128, S], BF16, tag="lk")
                        nc.scalar.activation(
                            lk[0:tl, :], psS[0:tl, 0:S], AF.Prelu,
                            scale=RSQ, alpha=0.2)
                        em = emp.tile([128, S], BF16, tag=f"em{ti}")
                        nc.scalar.activation(em[0:tl, :], lk[0:tl, :], AF.Exp)
                        nc.vector.tensor_tensor(
                            em[0:tl, :], em[0:tl, :], adjt[ti][0:tl, :],
                            ALU.mult)
                        ems.append(em)
                    for (si_, (s0, s1)) in enumerate(TT):
                        sl = s1 - s0
                        psR = psRp.tile([128, 8], FP32, tag="psR")
                        for (ti, (t0, t1)) in enumerate(TT):
                            tl = t1 - t0
                            nc.tensor.matmul(
                                psR[0:sl, 0:2], ems[ti][0:tl, s0:s1],
                                onesb[0:tl, :],
                                start=(ti == 0), stop=(ti == 2))
                        rs = rsp.tile([128, 1], FP32, tag="rs")
                        nc.vector.reciprocal(rs[0:sl, :], psR[0:sl, 0:1])
                        psA = psAp.tile([128, T_steps], FP32, tag="psA")
                        for (ti, (t0, t1)) in enumerate(TT):
                            tl = t1 - t0
                            nc.tensor.matmul(
                                psA[0:sl, :], ems[ti][0:tl, s0:s1],
                                v_sb5[0:tl, ti, h, b, :],
                                start=(ti == 0), stop=(ti == 2))
                        asm5 = asms[si_][:].rearrange(
                            "p (l hh) -> p l hh", hh=8)
                        nc.scalar.activation(
                            asm5[0:sl, :, h], psA[0:sl, :], AF.Prelu,
                            scale=rs[0:sl, :], alpha=0.2)
                for (si_, (s0, s1)) in enumerate(TT):
                    sl = s1 - s0
                    nc.sync.dma_start(
                        out_ext[b, s0:s1], asms[si_][0:sl, :].rearrange(
                            "p (l hh) -> p l hh", hh=8))

    return nc


# ------------------------------------------------------------------- host ---
def _prep(inputs, T_steps=T):
    import ml_dtypes
    bf16 = ml_dtypes.bfloat16

    x = np.asarray(inputs["x"], np.float32)          # [B,S,L,1]
    graph = np.asarray(inputs["graph"], np.float32)  # [S,S]

    shared = {}
    whhT = np.zeros((2, 8, 4, 128, 128), np.float32)
    wih = np.zeros((2, 8, 4, 128), np.float32)
    bias = np.zeros((2, 8, 4, 128), np.float32)
    for pidx, pre in enumerate(("q", "k")):
        W_ih = np.asarray(inputs[f"{pre}_Wih"], np.float32)   # [8,512,1]
        W_hh = np.asarray(inputs[f"{pre}_Whh"], np.float32)   # [8,512,128]
        b_ = (np.asarray(inputs[f"{pre}_bih"], np.float32)
              + np.asarray(inputs[f"{pre}_bhh"], np.float32))  # [8,512]
        for h in range(8):
            for g in range(4):
                sc = 2.0 if g == 2 else 1.0
                whhT[pidx, h, g] = sc * W_hh[h, g * 128:(g + 1) * 128, :].T
                wih[pidx, h, g] = sc * W_ih[h, g * 128:(g + 1) * 128, 0]
                bias[pidx, h, g] = sc * b_[h, g * 128:(g + 1) * 128]
    shared["whhT"] = whhT
    shared["wih"] = wih
    shared["bias"] = bias

    vW_ih = np.asarray(inputs["v_Wih"], np.float32)[:, :, 0]  # [8,4]
    vW_hh = np.asarray(inputs["v_Whh"], np.float32)[:, :, 0]  # [8,4]
    vb = (np.asarray(inputs["v_bih"], np.float32)
          + np.asarray(inputs["v_bhh"], np.float32))          # [8,4]
    vw = np.zeros((3, 4, 32), np.float32)
    for h in range(8):
        for b in range(NB):
            j = h * NB + b
            vw[0, :, j] = vW_ih[h]
            vw[1, :, j] = vW_hh[h]
            vw[2, :, j] = vb[h]
    shared["vw"] = vw

    A = ((graph + np.eye(S, dtype=np.float32)) != 0).astype(np.float32)
    adjT = np.zeros((3, 128, S), np.float32)
    for ti, (t0, t1) in enumerate(TT):
        adjT[ti, 0:t1 - t0] = A[t0:t1, :]
    shared["adjT"] = adjT.astype(bf16)
    shared["identb"] = np.eye(128, dtype=np.float32).astype(bf16)
    shared["identf"] = np.eye(32, dtype=np.float32)
    shared["ones"] = np.ones((128, 2), np.float32).astype(bf16)

    in_maps = []
    for core in range(NCORES):
        xc = x[core * NB:(core + 1) * NB, :, :, 0]   # [NB,S,L]
        xt = xc.transpose(2, 0, 1).reshape(T, N)[:T_steps]     # [T,N]
        x_rep = np.ascontiguousarray(
            np.broadcast_to(xt[:, None, :], (T_steps, 128, N))).astype(bf16)
        xvv = np.ascontiguousarray(
            np.broadcast_to(
                xc.transpose(2, 0, 1)[:T_steps, None, :, :],
                (T_steps, 8, NB, S)).reshape(T_steps, 32, S)).astype(bf16)
        m = dict(shared)
        m["x_rep"] = x_rep
        m["xv"] = xvv
        in_maps.append(m)
    return in_maps


def _run(inputs, T_steps=T, trace=False):
    import sys
    if "/root/problem" not in sys.path:
        sys.path.insert(0, "/root/problem")
    from concourse.bass_utils import run_bass_kernel_spmd

    key = T_steps
    if key not in _cache:
        _cache[key] = _build(T_steps)
    nc = _cache[key]
    in_maps = _prep(inputs, T_steps)
    res = run_bass_kernel_spmd(
        nc, in_maps, core_ids=list(range(NCORES)), trace=trace)
    out = np.concatenate([res.results[i]["out"] for i in range(NCORES)], axis=0)
    return out, res


def kernel(**inputs):
    out, _ = _run(inputs)
    return out.astype(np.float32)
